# revision 3
# baseline (speedup 1.0000x reference)
"""GCN layer (GraphConv + BN + dropout) as a Trainium2 Bass kernel, SPMD over 8 NeuronCores.

Strategy: shard destination nodes across cores, replicate feat; per core the
aggregation  agg[d] = sum_{e: dst(e)=d} norm_src[src]*norm_dst[d]*feat[src]
is computed by dma_gather of source rows + one-hot matmul accumulation in PSUM.
BatchNorm batch stats are all-reduced across cores.
"""
import sys

sys.path.insert(0, "/opt/trn_rl_repo")

import numpy as np

import concourse.bass as bass
import concourse.bacc as bacc
import concourse.mybir as mybir
import concourse.tile as tile
from concourse.masks import make_identity
from concourse.bass_utils import run_bass_kernel_spmd

NCORES = 8
P = 128
BANK = 32768          # rows addressable by int16 gather index
OPC = 4               # max chunks (of 128 rows) per dma_gather op
NQ = 4                # SWDGE queues to rotate gather ops over
BN_EPS = 1e-5


def _host_prep(feat, src, dst):
    """Partition edges by (core, dst-tile, src-bank); pad to 128-edge chunks with
    counts uniform across cores (max), producing per-core streams."""
    N, D = feat.shape
    rpc = N // NCORES                      # rows per core
    NT = (rpc + P - 1) // P                # dst tiles per core
    NB = (N + BANK - 1) // BANK            # src banks

    deg_out = np.bincount(src, minlength=N).astype(np.float64)
    deg_in = np.bincount(dst, minlength=N).astype(np.float64)
    norm_src = np.where(deg_out > 0, deg_out, 1.0) ** -0.5
    norm_dst = np.where(deg_in > 0, deg_in, 1.0) ** -0.5
    w_edge = (norm_src[src] * norm_dst[dst]).astype(np.float32)

    core_of = dst // rpc
    ldst = dst - core_of * rpc
    tile_of = ldst // P
    rel_of = (ldst % P).astype(np.float32)
    bank_of = src >> 15
    lsrc = (src & (BANK - 1)).astype(np.int16)
    seg_of = tile_of * NB + bank_of        # segment id within a core

    NSEG = NT * NB
    # per-core sorted edge streams + per-segment counts
    per_core = []
    L = np.zeros((NCORES, NSEG), np.int64)
    for c in range(NCORES):
        m = core_of == c
        order = np.argsort(seg_of[m], kind="stable")
        idx = np.nonzero(m)[0][order]
        per_core.append((lsrc[idx], rel_of[idx], w_edge[idx]))
        L[c] = np.bincount(seg_of[m], minlength=NSEG)

    n_chunks = (L.max(axis=0) + P - 1) // P        # [NSEG], uniform over cores
    seg_pad = n_chunks * P
    total_edges = int(seg_pad.sum())
    TC = total_edges // P                          # total chunks per core

    # ops: per segment, chunks grouped into dma_gather ops of <= OPC chunks
    ops = []  # (bank, nidx, chunk_base)
    k = 0
    for s in range(NSEG):
        nc_s = int(n_chunks[s])
        if nc_s == 0:
            continue
        b = s % NB
        done = 0
        while done < nc_s:
            take = min(OPC, nc_s - done)
            ops.append((b, take * P, k))
            k += take
            done += take
    assert k == TC

    # build padded per-core streams
    idx_w = total_edges // 16
    idx_all = np.zeros((NCORES, 128, idx_w), np.int16)
    rel_all = np.full((NCORES, P, TC), -1.0, np.float32)
    wts_all = np.zeros((NCORES, P, TC), np.float32)
    seg_off = np.zeros(NSEG + 1, np.int64)
    np.cumsum(seg_pad, out=seg_off[1:])
    for c in range(NCORES):
        ls, rel, wts = per_core[c]
        src_pad = np.zeros(total_edges, np.int16)
        rel_pad = np.full(total_edges, -1.0, np.float32)
        wts_pad = np.zeros(total_edges, np.float32)
        off = np.cumsum(np.concatenate([[0], L[c]]))[:-1]
        for s in range(NSEG):
            n = int(L[c, s])
            if n == 0:
                continue
            dstart = int(seg_off[s])
            src_pad[dstart : dstart + n] = ls[off[s] : off[s] + n]
            rel_pad[dstart : dstart + n] = rel[off[s] : off[s] + n]
            wts_pad[dstart : dstart + n] = wts[off[s] : off[s] + n]
        # chunk layout: chunk k edge j at [j, k]
        rel_all[c] = rel_pad.reshape(TC, P).T
        wts_all[c] = wts_pad.reshape(TC, P).T
        # idx packing per op: within an op, slot s' -> packed[s'%16, s'//16], x8 vertical
        pos = 0
        for (_b, nidx, kb) in ops:
            seg = src_pad[kb * P : kb * P + nidx]
            blk = seg.reshape(-1, 16).T          # [16, nidx/16]
            idx_all[c, :, pos : pos + nidx // 16] = np.tile(blk, (8, 1))
            pos += nidx // 16
        assert pos == idx_w

    meta = dict(N=N, D=D, rpc=rpc, NT=NT, NB=NB, TC=TC, idx_w=idx_w,
                n_chunks=n_chunks, ops=ops, total_edges=total_edges)
    return meta, idx_all, rel_all, wts_all


def _build_program(meta, repeat_main=1, use_collective=True):
    N, D = meta["N"], meta["D"]
    rpc, NT, NB, TC = meta["rpc"], meta["NT"], meta["NB"], meta["TC"]
    idx_w, ops, n_chunks = meta["idx_w"], meta["ops"], meta["n_chunks"]
    rpad = NT * P
    f32 = mybir.dt.float32

    nc = bacc.Bacc("TRN2", num_devices=NCORES, num_swdge_queues=NQ)
    feat = nc.declare_dram_parameter("feat", [N, D], f32, isOutput=False)
    idxs = nc.declare_dram_parameter("idxs", [128, idx_w], mybir.dt.int16, isOutput=False)
    drel = nc.declare_dram_parameter("drel", [P, TC], f32, isOutput=False)
    dwts = nc.declare_dram_parameter("dwts", [P, TC], f32, isOutput=False)
    maskT = nc.declare_dram_parameter("maskT", [D, rpad], f32, isOutput=False)
    Wp = nc.declare_dram_parameter("W", [D, D], f32, isOutput=False)
    bp = nc.declare_dram_parameter("b", [D], f32, isOutput=False)
    gp = nc.declare_dram_parameter("gamma", [D], f32, isOutput=False)
    btp = nc.declare_dram_parameter("beta", [D], f32, isOutput=False)
    yout = nc.declare_dram_parameter("y", [rpad, D], f32, isOutput=True)

    cc_in = nc.dram_tensor("cc_in", [D, 2], f32)
    cc_out = nc.dram_tensor("cc_out", [D, 2], f32, addr_space="Shared")

    iota_np = np.tile(np.arange(P, dtype=np.float32), (P, 1))
    iota_dram = nc.inline_tensor(iota_np, name="iota_const")

    # group ops by tile for the build loop
    ops_by_tile = [[] for _ in range(NT)]
    chunk_tile = np.repeat(np.arange(NT * NB) // NB, n_chunks)
    pos = 0
    for (b, nidx, kb) in ops:
        t = kb and int(chunk_tile[kb]) or int(chunk_tile[kb])
        ops_by_tile[int(chunk_tile[kb])].append((b, nidx, kb, pos))
        pos += nidx // 16

    with tile.TileContext(nc) as tc:
        with tc.tile_pool(name="consts", bufs=1) as cpool, \
             tc.tile_pool(name="streams", bufs=1) as stpool, \
             tc.tile_pool(name="hbuf", bufs=1) as hpool, \
             tc.tile_pool(name="gat", bufs=8) as gpool, \
             tc.tile_pool(name="sel", bufs=8) as spool, \
             tc.tile_pool(name="aggsb", bufs=4) as apool, \
             tc.tile_pool(name="sq", bufs=2) as qpool, \
             tc.tile_pool(name="ybuf", bufs=4) as ypool, \
             tc.tile_pool(name="msk", bufs=4) as mpool, \
             tc.tile_pool(name="stat", bufs=1) as tpool, \
             tc.tile_pool(name="apsum", bufs=4, space="PSUM") as apsum, \
             tc.tile_pool(name="hpsum", bufs=2, space="PSUM") as hpsum, \
             tc.tile_pool(name="tpsum", bufs=2, space="PSUM") as tpsum:

            identity = cpool.tile([P, P], f32)
            make_identity(nc, identity[:])
            iota_sb = cpool.tile([P, P], f32)
            nc.sync.dma_start(out=iota_sb[:], in_=iota_dram[:])
            W_sb = cpool.tile([D, D], f32)
            nc.sync.dma_start(out=W_sb[:], in_=Wp[:])
            b_sb = cpool.tile([D, 1], f32)
            nc.sync.dma_start(out=b_sb[:], in_=bp[:, None])
            g_sb = cpool.tile([D, 1], f32)
            nc.sync.dma_start(out=g_sb[:], in_=gp[:, None])
            bt_sb = cpool.tile([D, 1], f32)
            nc.sync.dma_start(out=bt_sb[:], in_=btp[:, None])

            idx_sb = stpool.tile([128, idx_w], mybir.dt.int16)
            nc.sync.dma_start(out=idx_sb[:], in_=idxs[:])
            drel_sb = stpool.tile([P, TC], f32)
            nc.sync.dma_start(out=drel_sb[:], in_=drel[:])
            dwts_sb = stpool.tile([P, TC], f32)
            nc.sync.dma_start(out=dwts_sb[:], in_=dwts[:])

            h_sb = hpool.tile([D, rpad], f32)
            sumbuf = tpool.tile([D, NT], f32)
            sqbuf = tpool.tile([D, NT], f32)

            opi = 0
            for rpt in range(repeat_main):
              for t in range(NT):
                agg_ps = apsum.tile([D, P], f32, tag="agg", name=f"agg{t}")
                nchunks_t = sum(nidx // P for (_b, nidx, _kb, _po) in ops_by_tile[t])
                done = 0
                for (bk, nidx, kb, po) in ops_by_tile[t]:
                    g = gpool.tile([128, nidx], f32, tag="g", name=f"g{t}_{kb}")
                    bank_lo = bk * BANK
                    bank_hi = min(bank_lo + BANK, N)
                    nc.gpsimd.dma_gather(
                        g[:].rearrange("p (c e) -> p c e", e=D),
                        feat[bank_lo:bank_hi, :],
                        idx_sb[:, po : po + nidx // 16],
                        nidx, nidx, D,
                        single_packet=True, queue_num=opi % NQ,
                    )
                    opi += 1
                    for j in range(nidx // P):
                        k = kb + j
                        S = spool.tile([P, P], f32, tag="S", name=f"S{k}")
                        nc.vector.tensor_scalar(
                            out=S[:], in0=iota_sb[:],
                            scalar1=drel_sb[:, k : k + 1],
                            scalar2=dwts_sb[:, k : k + 1],
                            op0=mybir.AluOpType.is_equal,
                            op1=mybir.AluOpType.mult,
                        )
                        nc.tensor.matmul(
                            out=agg_ps[:],
                            lhsT=g[:, j * D : (j + 1) * D],
                            rhs=S[:],
                            start=(done == 0), stop=(done == nchunks_t - 1),
                        )
                        done += 1

                aggT = apool.tile([D, P], f32, tag="aggT", name=f"aggT{t}")
                nc.vector.tensor_copy(out=aggT[:], in_=agg_ps[:])
                hT_ps = hpsum.tile([D, P], f32, tag="hT", name=f"hT{t}")
                nc.tensor.matmul(out=hT_ps[:], lhsT=W_sb[:], rhs=aggT[:],
                                 start=True, stop=True)
                hslice = h_sb[:, t * P : (t + 1) * P]
                nc.scalar.activation(
                    out=hslice, in_=hT_ps[:],
                    func=mybir.ActivationFunctionType.Relu,
                    bias=b_sb[:], accum_out=sumbuf[:, t : t + 1],
                )
                sq = qpool.tile([D, P], f32, tag="sq", name=f"sq{t}")
                nc.scalar.activation(
                    out=sq[:], in_=hslice,
                    func=mybir.ActivationFunctionType.Square,
                    accum_out=sqbuf[:, t : t + 1],
                )

            # BN stats: reduce per-tile partial sums, all-reduce across cores
            stats = tpool.tile([D, 2], f32)
            nc.vector.reduce_sum(out=stats[:, 0:1], in_=sumbuf[:], axis=mybir.AxisListType.X)
            nc.vector.reduce_sum(out=stats[:, 1:2], in_=sqbuf[:], axis=mybir.AxisListType.X)
            nc.sync.dma_start(out=cc_in[:], in_=stats[:])
            if use_collective:
                nc.gpsimd.collective_compute(
                    "AllReduce", mybir.AluOpType.add,
                    replica_groups=[list(range(NCORES))],
                    ins=[cc_in[:]], outs=[cc_out[:]],
                )
            else:
                nc.sync.dma_start(out=cc_out[:], in_=cc_in[:])
            gstats = tpool.tile([D, 2], f32)
            nc.sync.dma_start(out=gstats[:], in_=cc_out[:])

            inv_n = 1.0 / float(N)
            mean = tpool.tile([D, 1], f32)
            nc.vector.tensor_scalar_mul(out=mean[:], in0=gstats[:, 0:1], scalar1=inv_n)
            ex2 = tpool.tile([D, 1], f32)
            nc.vector.tensor_scalar_mul(out=ex2[:], in0=gstats[:, 1:2], scalar1=inv_n)
            m2 = tpool.tile([D, 1], f32)
            nc.vector.tensor_mul(out=m2[:], in0=mean[:], in1=mean[:])
            vare = tpool.tile([D, 1], f32)
            nc.vector.tensor_sub(out=vare[:], in0=ex2[:], in1=m2[:])
            nc.vector.tensor_scalar_add(out=vare[:], in0=vare[:], scalar1=BN_EPS)
            rvar = tpool.tile([D, 1], f32)
            nc.vector.reciprocal(out=rvar[:], in_=vare[:])
            rstd = tpool.tile([D, 1], f32)
            nc.scalar.activation(out=rstd[:], in_=rvar[:],
                                 func=mybir.ActivationFunctionType.Sqrt)
            Avec = tpool.tile([D, 1], f32)
            nc.vector.tensor_mul(out=Avec[:], in0=g_sb[:], in1=rstd[:])
            mA = tpool.tile([D, 1], f32)
            nc.vector.tensor_mul(out=mA[:], in0=mean[:], in1=Avec[:])
            Bvec = tpool.tile([D, 1], f32)
            nc.vector.tensor_sub(out=Bvec[:], in0=bt_sb[:], in1=mA[:])

            # apply BN + dropout mask, transpose back, write out
            for rpt in range(repeat_main):
              for t in range(NT):
                yt = ypool.tile([D, P], f32, tag="yt", name=f"yt{t}")
                nc.vector.tensor_scalar(
                    out=yt[:], in0=h_sb[:, t * P : (t + 1) * P],
                    scalar1=Avec[:], scalar2=Bvec[:],
                    op0=mybir.AluOpType.mult, op1=mybir.AluOpType.add,
                )
                mk = mpool.tile([D, P], f32, tag="mk", name=f"mk{t}")
                nc.sync.dma_start(out=mk[:], in_=maskT[:, t * P : (t + 1) * P])
                nc.vector.tensor_mul(out=yt[:], in0=yt[:], in1=mk[:])
                tp = tpsum.tile([P, D], f32, tag="tp", name=f"tp{t}")
                nc.tensor.transpose(out=tp[:], in_=yt[:], identity=identity[:])
                yo = ypool.tile([P, D], f32, tag="yo", name=f"yo{t}")
                nc.scalar.copy(out=yo[:], in_=tp[:])
                nc.sync.dma_start(out=yout[t * P : (t + 1) * P, :], in_=yo[:])

    nc.compile()
    return nc


def kernel(feat, src, dst, W, b, gamma, beta):
    feat = np.ascontiguousarray(np.asarray(feat, np.float32))
    src = np.asarray(src).astype(np.int64)
    dst = np.asarray(dst).astype(np.int64)
    N, D = feat.shape
    assert D == 128 and N % NCORES == 0

    meta, idx_all, rel_all, wts_all = _host_prep(feat, src, dst)
    rpc, NT = meta["rpc"], meta["NT"]
    rpad = NT * P

    # dropout mask (deterministic, matches reference)
    import jax
    with jax.default_device(jax.devices("cpu")[0]):
        keep = jax.random.bernoulli(jax.random.key(42), 0.5, (N, D))
        maskv = (np.asarray(keep).astype(np.float32)) * 2.0

    nc = _build_program(meta)

    in_maps = []
    for c in range(NCORES):
        mrows = np.zeros((rpad, D), np.float32)
        mrows[:rpc] = maskv[c * rpc : (c + 1) * rpc]
        in_maps.append({
            "feat": feat,
            "idxs": np.ascontiguousarray(idx_all[c]),
            "drel": np.ascontiguousarray(rel_all[c]),
            "dwts": np.ascontiguousarray(wts_all[c]),
            "maskT": np.ascontiguousarray(mrows.T),
            "W": np.ascontiguousarray(np.asarray(W, np.float32)),
            "b": np.ascontiguousarray(np.asarray(b, np.float32)),
            "gamma": np.ascontiguousarray(np.asarray(gamma, np.float32)),
            "beta": np.ascontiguousarray(np.asarray(beta, np.float32)),
        })

    res = run_bass_kernel_spmd(nc, in_maps, list(range(NCORES)))
    out = np.concatenate([res.results[c]["y"][:rpc] for c in range(NCORES)], axis=0)
    return out


# revision 4
# speedup vs baseline: 1.0903x; 1.0903x over previous
"""GCN layer (GraphConv + BN + dropout) as a Trainium2 Bass kernel, SPMD over 8 NeuronCores.

Strategy: shard destination nodes across cores, replicate feat; per core the
aggregation  agg[d] = sum_{e: dst(e)=d} norm_src[src]*norm_dst[d]*feat[src]
is computed by dma_gather of source rows + one-hot matmul accumulation in PSUM.
BatchNorm batch stats are all-reduced across cores.
"""
import sys

sys.path.insert(0, "/opt/trn_rl_repo")

import numpy as np

import concourse.bass as bass
import concourse.bacc as bacc
import concourse.mybir as mybir
import concourse.tile as tile
from concourse.masks import make_identity
from concourse.bass_utils import run_bass_kernel_spmd

NCORES = 8
P = 128
BANK = 32768          # rows addressable by int16 gather index
OPC = 4               # max chunks (of 128 rows) per dma_gather op
NQ = 4                # SWDGE queues to rotate gather ops over
BN_EPS = 1e-5


def _host_prep(feat, src, dst):
    """Partition edges by (core, dst-tile, src-bank); pad to 128-edge chunks with
    counts uniform across cores (max), producing per-core streams."""
    N, D = feat.shape
    rpc = N // NCORES                      # rows per core
    NT = (rpc + P - 1) // P                # dst tiles per core
    NB = (N + BANK - 1) // BANK            # src banks

    deg_out = np.bincount(src, minlength=N).astype(np.float64)
    deg_in = np.bincount(dst, minlength=N).astype(np.float64)
    norm_src = np.where(deg_out > 0, deg_out, 1.0) ** -0.5
    norm_dst = np.where(deg_in > 0, deg_in, 1.0) ** -0.5
    w_edge = (norm_src[src] * norm_dst[dst]).astype(np.float32)

    core_of = dst // rpc
    ldst = dst - core_of * rpc
    tile_of = ldst // P
    rel_of = (ldst % P).astype(np.float32)
    bank_of = src >> 15
    lsrc = (src & (BANK - 1)).astype(np.int16)
    seg_of = tile_of * NB + bank_of        # segment id within a core

    NSEG = NT * NB
    # per-core sorted edge streams + per-segment counts
    per_core = []
    L = np.zeros((NCORES, NSEG), np.int64)
    for c in range(NCORES):
        m = core_of == c
        order = np.argsort(seg_of[m], kind="stable")
        idx = np.nonzero(m)[0][order]
        per_core.append((lsrc[idx], rel_of[idx], w_edge[idx]))
        L[c] = np.bincount(seg_of[m], minlength=NSEG)

    n_chunks = (L.max(axis=0) + P - 1) // P        # [NSEG], uniform over cores
    seg_pad = n_chunks * P
    total_edges = int(seg_pad.sum())
    TC = total_edges // P                          # total chunks per core

    # ops: per segment, chunks grouped into dma_gather ops of <= OPC chunks
    ops = []  # (bank, nidx, chunk_base)
    k = 0
    for s in range(NSEG):
        nc_s = int(n_chunks[s])
        if nc_s == 0:
            continue
        b = s % NB
        done = 0
        while done < nc_s:
            take = min(OPC, nc_s - done)
            ops.append((b, take * P, k))
            k += take
            done += take
    assert k == TC

    # build padded per-core streams
    idx_w = total_edges // 16
    idx_all = np.zeros((NCORES, 128, idx_w), np.int16)
    rel_all = np.full((NCORES, P, TC), -1.0, np.float32)
    wts_all = np.zeros((NCORES, P, TC), np.float32)
    seg_off = np.zeros(NSEG + 1, np.int64)
    np.cumsum(seg_pad, out=seg_off[1:])
    for c in range(NCORES):
        ls, rel, wts = per_core[c]
        src_pad = np.zeros(total_edges, np.int16)
        rel_pad = np.full(total_edges, -1.0, np.float32)
        wts_pad = np.zeros(total_edges, np.float32)
        off = np.cumsum(np.concatenate([[0], L[c]]))[:-1]
        for s in range(NSEG):
            n = int(L[c, s])
            if n == 0:
                continue
            dstart = int(seg_off[s])
            src_pad[dstart : dstart + n] = ls[off[s] : off[s] + n]
            rel_pad[dstart : dstart + n] = rel[off[s] : off[s] + n]
            wts_pad[dstart : dstart + n] = wts[off[s] : off[s] + n]
        # chunk layout: chunk k edge j at [j, k]
        rel_all[c] = rel_pad.reshape(TC, P).T
        wts_all[c] = wts_pad.reshape(TC, P).T
        # idx packing per op: within an op, slot s' -> packed[s'%16, s'//16], x8 vertical
        pos = 0
        for (_b, nidx, kb) in ops:
            seg = src_pad[kb * P : kb * P + nidx]
            blk = seg.reshape(-1, 16).T          # [16, nidx/16]
            idx_all[c, :, pos : pos + nidx // 16] = np.tile(blk, (8, 1))
            pos += nidx // 16
        assert pos == idx_w

    meta = dict(N=N, D=D, rpc=rpc, NT=NT, NB=NB, TC=TC, idx_w=idx_w,
                n_chunks=n_chunks, ops=ops, total_edges=total_edges)
    return meta, idx_all, rel_all, wts_all


def _build_program(meta, repeat_main=1, use_collective=True, stages=5):
    # stages: 1=gather only, 2=+sbuild, 3=+agg matmuls, 4=+stageB, 5=+apply
    N, D = meta["N"], meta["D"]
    rpc, NT, NB, TC = meta["rpc"], meta["NT"], meta["NB"], meta["TC"]
    idx_w, ops, n_chunks = meta["idx_w"], meta["ops"], meta["n_chunks"]
    rpad = NT * P
    f32 = mybir.dt.float32

    nc = bacc.Bacc("TRN2", num_devices=NCORES, num_swdge_queues=NQ)
    feat = nc.declare_dram_parameter("feat", [N, D], f32, isOutput=False)
    idxs = nc.declare_dram_parameter("idxs", [128, idx_w], mybir.dt.int16, isOutput=False)
    drel = nc.declare_dram_parameter("drel", [P, TC], f32, isOutput=False)
    dwts = nc.declare_dram_parameter("dwts", [P, TC], f32, isOutput=False)
    maskT = nc.declare_dram_parameter("maskT", [D, rpad], f32, isOutput=False)
    Wp = nc.declare_dram_parameter("W", [D, D], f32, isOutput=False)
    bp = nc.declare_dram_parameter("b", [D], f32, isOutput=False)
    gp = nc.declare_dram_parameter("gamma", [D], f32, isOutput=False)
    btp = nc.declare_dram_parameter("beta", [D], f32, isOutput=False)
    yout = nc.declare_dram_parameter("y", [rpad, D], f32, isOutput=True)

    cc_in = nc.dram_tensor("cc_in", [D, 2], f32)
    cc_out = nc.dram_tensor("cc_out", [D, 2], f32, addr_space="Shared")

    iota_np = np.tile(np.arange(P, dtype=np.float32), (P, 1))
    iota_dram = nc.inline_tensor(iota_np, name="iota_const")

    # group ops by tile for the build loop
    ops_by_tile = [[] for _ in range(NT)]
    chunk_tile = np.repeat(np.arange(NT * NB) // NB, n_chunks)
    pos = 0
    for (b, nidx, kb) in ops:
        t = kb and int(chunk_tile[kb]) or int(chunk_tile[kb])
        ops_by_tile[int(chunk_tile[kb])].append((b, nidx, kb, pos))
        pos += nidx // 16

    with tile.TileContext(nc) as tc:
        with tc.tile_pool(name="consts", bufs=1) as cpool, \
             tc.tile_pool(name="streams", bufs=1) as stpool, \
             tc.tile_pool(name="hbuf", bufs=1) as hpool, \
             tc.tile_pool(name="gat", bufs=8) as gpool, \
             tc.tile_pool(name="sel", bufs=8) as spool, \
             tc.tile_pool(name="aggsb", bufs=4) as apool, \
             tc.tile_pool(name="sq", bufs=2) as qpool, \
             tc.tile_pool(name="ybuf", bufs=4) as ypool, \
             tc.tile_pool(name="msk", bufs=4) as mpool, \
             tc.tile_pool(name="stat", bufs=1) as tpool, \
             tc.tile_pool(name="apsum", bufs=4, space="PSUM") as apsum, \
             tc.tile_pool(name="hpsum", bufs=2, space="PSUM") as hpsum, \
             tc.tile_pool(name="tpsum", bufs=2, space="PSUM") as tpsum:

            identity = cpool.tile([P, P], f32)
            make_identity(nc, identity[:])
            iota_sb = cpool.tile([P, P], f32)
            nc.sync.dma_start(out=iota_sb[:], in_=iota_dram[:])
            W_sb = cpool.tile([D, D], f32)
            nc.sync.dma_start(out=W_sb[:], in_=Wp[:])
            b_sb = cpool.tile([D, 1], f32)
            nc.sync.dma_start(out=b_sb[:], in_=bp[:, None])
            g_sb = cpool.tile([D, 1], f32)
            nc.sync.dma_start(out=g_sb[:], in_=gp[:, None])
            bt_sb = cpool.tile([D, 1], f32)
            nc.sync.dma_start(out=bt_sb[:], in_=btp[:, None])

            idx_sb = stpool.tile([128, idx_w], mybir.dt.int16)
            nc.sync.dma_start(out=idx_sb[:], in_=idxs[:])
            drel_sb = stpool.tile([P, TC], f32)
            nc.sync.dma_start(out=drel_sb[:], in_=drel[:])
            dwts_sb = stpool.tile([P, TC], f32)
            nc.sync.dma_start(out=dwts_sb[:], in_=dwts[:])

            h_sb = hpool.tile([D, rpad], f32)
            sumbuf = tpool.tile([D, NT], f32)
            sqbuf = tpool.tile([D, NT], f32)

            opi = 0
            for rpt in range(repeat_main):
              for t in range(NT):
                agg_ps = apsum.tile([D, P], f32, tag="agg", name=f"agg{t}")
                nchunks_t = sum(nidx // P for (_b, nidx, _kb, _po) in ops_by_tile[t])
                done = 0
                for (bk, nidx, kb, po) in ops_by_tile[t]:
                    g = gpool.tile([128, nidx], f32, tag="g", name=f"g{t}_{kb}")
                    bank_lo = bk * BANK
                    bank_hi = min(bank_lo + BANK, N)
                    nc.gpsimd.dma_gather(
                        g[:].rearrange("p (c e) -> p c e", e=D),
                        feat[bank_lo:bank_hi, :],
                        idx_sb[:, po : po + nidx // 16],
                        nidx, nidx, D,
                        single_packet=True, queue_num=opi % NQ,
                    )
                    opi += 1
                    if stages == 1:
                        nc.vector.tensor_add(out=sumbuf[:, t:t+1], in0=sumbuf[:, t:t+1], in1=g[:, 0:1])
                        done += nidx // P
                        continue
                    for j in range(nidx // P):
                        k = kb + j
                        S = spool.tile([P, P], f32, tag="S", name=f"S{k}")
                        nc.vector.tensor_scalar(
                            out=S[:], in0=iota_sb[:],
                            scalar1=drel_sb[:, k : k + 1],
                            scalar2=dwts_sb[:, k : k + 1],
                            op0=mybir.AluOpType.is_equal,
                            op1=mybir.AluOpType.mult,
                        )
                        if stages >= 3:
                            nc.tensor.matmul(
                                out=agg_ps[:],
                                lhsT=g[:, j * D : (j + 1) * D],
                                rhs=S[:],
                                start=(done == 0), stop=(done == nchunks_t - 1),
                            )
                        else:
                            nc.vector.tensor_add(out=sumbuf[:, t:t+1], in0=sumbuf[:, t:t+1], in1=S[:, 0:1])
                        done += 1

                if stages < 4:
                    continue
                aggT = apool.tile([D, P], f32, tag="aggT", name=f"aggT{t}")
                nc.vector.tensor_copy(out=aggT[:], in_=agg_ps[:])
                hT_ps = hpsum.tile([D, P], f32, tag="hT", name=f"hT{t}")
                nc.tensor.matmul(out=hT_ps[:], lhsT=W_sb[:], rhs=aggT[:],
                                 start=True, stop=True)
                hslice = h_sb[:, t * P : (t + 1) * P]
                nc.scalar.activation(
                    out=hslice, in_=hT_ps[:],
                    func=mybir.ActivationFunctionType.Relu,
                    bias=b_sb[:], accum_out=sumbuf[:, t : t + 1],
                )
                sq = qpool.tile([D, P], f32, tag="sq", name=f"sq{t}")
                nc.scalar.activation(
                    out=sq[:], in_=hslice,
                    func=mybir.ActivationFunctionType.Square,
                    accum_out=sqbuf[:, t : t + 1],
                )

            # BN stats: reduce per-tile partial sums, all-reduce across cores
            if stages < 4:
                nc.vector.tensor_copy(out=h_sb[:, 0:NT], in_=sumbuf[:])
            stats = tpool.tile([D, 2], f32)
            nc.vector.reduce_sum(out=stats[:, 0:1], in_=sumbuf[:], axis=mybir.AxisListType.X)
            nc.vector.reduce_sum(out=stats[:, 1:2], in_=sqbuf[:], axis=mybir.AxisListType.X)
            nc.sync.dma_start(out=cc_in[:], in_=stats[:])
            if use_collective:
                nc.gpsimd.collective_compute(
                    "AllReduce", mybir.AluOpType.add,
                    replica_groups=[list(range(NCORES))],
                    ins=[cc_in[:]], outs=[cc_out[:]],
                )
            else:
                nc.sync.dma_start(out=cc_out[:], in_=cc_in[:])
            gstats = tpool.tile([D, 2], f32)
            nc.sync.dma_start(out=gstats[:], in_=cc_out[:])

            inv_n = 1.0 / float(N)
            mean = tpool.tile([D, 1], f32)
            nc.vector.tensor_scalar_mul(out=mean[:], in0=gstats[:, 0:1], scalar1=inv_n)
            ex2 = tpool.tile([D, 1], f32)
            nc.vector.tensor_scalar_mul(out=ex2[:], in0=gstats[:, 1:2], scalar1=inv_n)
            m2 = tpool.tile([D, 1], f32)
            nc.vector.tensor_mul(out=m2[:], in0=mean[:], in1=mean[:])
            vare = tpool.tile([D, 1], f32)
            nc.vector.tensor_sub(out=vare[:], in0=ex2[:], in1=m2[:])
            nc.vector.tensor_scalar_add(out=vare[:], in0=vare[:], scalar1=BN_EPS)
            rvar = tpool.tile([D, 1], f32)
            nc.vector.reciprocal(out=rvar[:], in_=vare[:])
            rstd = tpool.tile([D, 1], f32)
            nc.scalar.activation(out=rstd[:], in_=rvar[:],
                                 func=mybir.ActivationFunctionType.Sqrt)
            Avec = tpool.tile([D, 1], f32)
            nc.vector.tensor_mul(out=Avec[:], in0=g_sb[:], in1=rstd[:])
            mA = tpool.tile([D, 1], f32)
            nc.vector.tensor_mul(out=mA[:], in0=mean[:], in1=Avec[:])
            Bvec = tpool.tile([D, 1], f32)
            nc.vector.tensor_sub(out=Bvec[:], in0=bt_sb[:], in1=mA[:])

            # apply BN + dropout mask, transpose back, write out
            for rpt in range(repeat_main if stages >= 5 else 0):
              for t in range(NT):
                yt = ypool.tile([D, P], f32, tag="yt", name=f"yt{t}")
                nc.vector.tensor_scalar(
                    out=yt[:], in0=h_sb[:, t * P : (t + 1) * P],
                    scalar1=Avec[:], scalar2=Bvec[:],
                    op0=mybir.AluOpType.mult, op1=mybir.AluOpType.add,
                )
                mk = mpool.tile([D, P], f32, tag="mk", name=f"mk{t}")
                nc.sync.dma_start(out=mk[:], in_=maskT[:, t * P : (t + 1) * P])
                nc.vector.tensor_mul(out=yt[:], in0=yt[:], in1=mk[:])
                tp = tpsum.tile([P, D], f32, tag="tp", name=f"tp{t}")
                nc.tensor.transpose(out=tp[:], in_=yt[:], identity=identity[:])
                yo = ypool.tile([P, D], f32, tag="yo", name=f"yo{t}")
                nc.scalar.copy(out=yo[:], in_=tp[:])
                nc.sync.dma_start(out=yout[t * P : (t + 1) * P, :], in_=yo[:])

    nc.compile()
    return nc


def kernel(feat, src, dst, W, b, gamma, beta):
    feat = np.ascontiguousarray(np.asarray(feat, np.float32))
    src = np.asarray(src).astype(np.int64)
    dst = np.asarray(dst).astype(np.int64)
    N, D = feat.shape
    assert D == 128 and N % NCORES == 0

    meta, idx_all, rel_all, wts_all = _host_prep(feat, src, dst)
    rpc, NT = meta["rpc"], meta["NT"]
    rpad = NT * P

    # dropout mask (deterministic, matches reference)
    import jax
    with jax.default_device(jax.devices("cpu")[0]):
        keep = jax.random.bernoulli(jax.random.key(42), 0.5, (N, D))
        maskv = (np.asarray(keep).astype(np.float32)) * 2.0

    nc = _build_program(meta)

    in_maps = []
    for c in range(NCORES):
        mrows = np.zeros((rpad, D), np.float32)
        mrows[:rpc] = maskv[c * rpc : (c + 1) * rpc]
        in_maps.append({
            "feat": feat,
            "idxs": np.ascontiguousarray(idx_all[c]),
            "drel": np.ascontiguousarray(rel_all[c]),
            "dwts": np.ascontiguousarray(wts_all[c]),
            "maskT": np.ascontiguousarray(mrows.T),
            "W": np.ascontiguousarray(np.asarray(W, np.float32)),
            "b": np.ascontiguousarray(np.asarray(b, np.float32)),
            "gamma": np.ascontiguousarray(np.asarray(gamma, np.float32)),
            "beta": np.ascontiguousarray(np.asarray(beta, np.float32)),
        })

    res = run_bass_kernel_spmd(nc, in_maps, list(range(NCORES)))
    out = np.concatenate([res.results[c]["y"][:rpc] for c in range(NCORES)], axis=0)
    return out


# revision 5
# speedup vs baseline: 4.6346x; 4.2508x over previous
"""GCN layer (GraphConv + BN + dropout) as a Trainium2 Bass kernel, SPMD over 8 NeuronCores.

Strategy: shard destination nodes across cores, replicate feat; per core the
aggregation  agg[d] = sum_{e: dst(e)=d} norm_src[src]*norm_dst[d]*feat[src]
is computed by dma_gather of source rows + one-hot matmul accumulation in PSUM.
BatchNorm batch stats are all-reduced across cores.
"""
import sys

sys.path.insert(0, "/opt/trn_rl_repo")

import numpy as np

import concourse.bass as bass
import concourse.bacc as bacc
import concourse.mybir as mybir
import concourse.tile as tile
from concourse.masks import make_identity
from concourse.bass_utils import run_bass_kernel_spmd

NCORES = 8
P = 128
BANK = 32768          # rows addressable by int16 gather index
OPC = 4               # max chunks (of 128 rows) per dma_gather op
NQ = 4                # SWDGE queues to rotate gather ops over
BN_EPS = 1e-5


def _host_prep(feat, src, dst):
    """Partition edges by (core, dst-tile, src-bank); pad to 128-edge chunks with
    counts uniform across cores (max), producing per-core streams."""
    N, D = feat.shape
    rpc = N // NCORES                      # rows per core
    NT = (rpc + P - 1) // P                # dst tiles per core
    NB = (N + BANK - 1) // BANK            # src banks

    deg_out = np.bincount(src, minlength=N).astype(np.float64)
    deg_in = np.bincount(dst, minlength=N).astype(np.float64)
    norm_src = np.where(deg_out > 0, deg_out, 1.0) ** -0.5
    norm_dst = np.where(deg_in > 0, deg_in, 1.0) ** -0.5
    w_edge = (norm_src[src] * norm_dst[dst]).astype(np.float32)

    core_of = dst // rpc
    ldst = dst - core_of * rpc
    tile_of = ldst // P
    rel_of = (ldst % P).astype(np.float32)
    bank_of = src >> 15
    lsrc = (src & (BANK - 1)).astype(np.int16)
    seg_of = tile_of * NB + bank_of        # segment id within a core

    NSEG = NT * NB
    # per-core sorted edge streams + per-segment counts
    per_core = []
    L = np.zeros((NCORES, NSEG), np.int64)
    for c in range(NCORES):
        m = core_of == c
        order = np.argsort(seg_of[m], kind="stable")
        idx = np.nonzero(m)[0][order]
        per_core.append((lsrc[idx], rel_of[idx], w_edge[idx]))
        L[c] = np.bincount(seg_of[m], minlength=NSEG)

    n_chunks = (L.max(axis=0) + P - 1) // P        # [NSEG], uniform over cores
    seg_pad = n_chunks * P
    total_edges = int(seg_pad.sum())
    TC = total_edges // P                          # total chunks per core

    # ops: per segment, chunks grouped into dma_gather ops of <= OPC chunks
    ops = []  # (bank, nidx, chunk_base)
    k = 0
    for s in range(NSEG):
        nc_s = int(n_chunks[s])
        if nc_s == 0:
            continue
        b = s % NB
        done = 0
        while done < nc_s:
            take = min(OPC, nc_s - done)
            ops.append((b, take * P, k))
            k += take
            done += take
    assert k == TC

    # build padded per-core streams
    idx_w = total_edges // 16
    idx_all = np.zeros((NCORES, 128, idx_w), np.int16)
    rel_all = np.full((NCORES, P, TC), -1.0, np.float32)
    wts_all = np.zeros((NCORES, P, TC), np.float32)
    seg_off = np.zeros(NSEG + 1, np.int64)
    np.cumsum(seg_pad, out=seg_off[1:])
    for c in range(NCORES):
        ls, rel, wts = per_core[c]
        src_pad = np.zeros(total_edges, np.int16)
        rel_pad = np.full(total_edges, -1.0, np.float32)
        wts_pad = np.zeros(total_edges, np.float32)
        off = np.cumsum(np.concatenate([[0], L[c]]))[:-1]
        for s in range(NSEG):
            n = int(L[c, s])
            if n == 0:
                continue
            dstart = int(seg_off[s])
            src_pad[dstart : dstart + n] = ls[off[s] : off[s] + n]
            rel_pad[dstart : dstart + n] = rel[off[s] : off[s] + n]
            wts_pad[dstart : dstart + n] = wts[off[s] : off[s] + n]
        # chunk layout: chunk k edge j at [j, k]
        rel_all[c] = rel_pad.reshape(TC, P).T
        wts_all[c] = wts_pad.reshape(TC, P).T
        # idx packing per op: within an op, slot s' -> packed[s'%16, s'//16], x8 vertical
        pos = 0
        for (_b, nidx, kb) in ops:
            seg = src_pad[kb * P : kb * P + nidx]
            blk = seg.reshape(-1, 16).T          # [16, nidx/16]
            idx_all[c, :, pos : pos + nidx // 16] = np.tile(blk, (8, 1))
            pos += nidx // 16
        assert pos == idx_w

    meta = dict(N=N, D=D, rpc=rpc, NT=NT, NB=NB, TC=TC, idx_w=idx_w,
                n_chunks=n_chunks, ops=ops, total_edges=total_edges)
    return meta, idx_all, rel_all, wts_all


def _build_program(meta, repeat_main=1, use_collective=True, stages=5):
    # stages: 1=gather only, 2=+sbuild, 3=+agg matmuls, 4=+stageB, 5=+apply
    N, D = meta["N"], meta["D"]
    rpc, NT, NB, TC = meta["rpc"], meta["NT"], meta["NB"], meta["TC"]
    idx_w, ops, n_chunks = meta["idx_w"], meta["ops"], meta["n_chunks"]
    rpad = NT * P
    f32 = mybir.dt.float32

    nc = bacc.Bacc("TRN2", num_devices=NCORES, num_swdge_queues=NQ)
    feat = nc.declare_dram_parameter("feat", [N, D], f32, isOutput=False)
    idxs = nc.declare_dram_parameter("idxs", [128, idx_w], mybir.dt.int16, isOutput=False)
    drel = nc.declare_dram_parameter("drel", [P, TC], f32, isOutput=False)
    dwts = nc.declare_dram_parameter("dwts", [P, TC], f32, isOutput=False)
    maskT = nc.declare_dram_parameter("maskT", [D, rpad], f32, isOutput=False)
    Wp = nc.declare_dram_parameter("W", [D, D], f32, isOutput=False)
    bp = nc.declare_dram_parameter("b", [D], f32, isOutput=False)
    gp = nc.declare_dram_parameter("gamma", [D], f32, isOutput=False)
    btp = nc.declare_dram_parameter("beta", [D], f32, isOutput=False)
    yout = nc.declare_dram_parameter("y", [rpad, D], f32, isOutput=True)

    cc_in = nc.dram_tensor("cc_in", [D, 2], f32)
    cc_out = nc.dram_tensor("cc_out", [D, 2], f32, addr_space="Shared")

    iota_np = np.tile(np.arange(P, dtype=np.float32), (P, 1))
    iota_dram = nc.inline_tensor(iota_np, name="iota_const")

    # group ops by tile for the build loop
    ops_by_tile = [[] for _ in range(NT)]
    chunk_tile = np.repeat(np.arange(NT * NB) // NB, n_chunks)
    pos = 0
    for (b, nidx, kb) in ops:
        t = kb and int(chunk_tile[kb]) or int(chunk_tile[kb])
        ops_by_tile[int(chunk_tile[kb])].append((b, nidx, kb, pos))
        pos += nidx // 16

    with tile.TileContext(nc) as tc:
        with tc.tile_pool(name="consts", bufs=1) as cpool, \
             tc.tile_pool(name="streams", bufs=1) as stpool, \
             tc.tile_pool(name="hbuf", bufs=1) as hpool, \
             tc.tile_pool(name="gat", bufs=8) as gpool, \
             tc.tile_pool(name="sel", bufs=8) as spool, \
             tc.tile_pool(name="aggsb", bufs=4) as apool, \
             tc.tile_pool(name="sq", bufs=2) as qpool, \
             tc.tile_pool(name="ybuf", bufs=4) as ypool, \
             tc.tile_pool(name="msk", bufs=4) as mpool, \
             tc.tile_pool(name="stat", bufs=1) as tpool, \
             tc.tile_pool(name="apsum", bufs=4, space="PSUM") as apsum, \
             tc.tile_pool(name="hpsum", bufs=2, space="PSUM") as hpsum, \
             tc.tile_pool(name="tpsum", bufs=2, space="PSUM") as tpsum:

            identity = cpool.tile([P, P], f32)
            make_identity(nc, identity[:])
            iota_sb = cpool.tile([P, P], f32)
            nc.sync.dma_start(out=iota_sb[:], in_=iota_dram[:])
            W_sb = cpool.tile([D, D], f32)
            nc.sync.dma_start(out=W_sb[:], in_=Wp[:])
            b_sb = cpool.tile([D, 1], f32)
            nc.sync.dma_start(out=b_sb[:], in_=bp[:, None])
            g_sb = cpool.tile([D, 1], f32)
            nc.sync.dma_start(out=g_sb[:], in_=gp[:, None])
            bt_sb = cpool.tile([D, 1], f32)
            nc.sync.dma_start(out=bt_sb[:], in_=btp[:, None])

            idx_sb = stpool.tile([128, idx_w], mybir.dt.int16)
            nc.sync.dma_start(out=idx_sb[:], in_=idxs[:])
            drel_sb = stpool.tile([P, TC], f32)
            nc.sync.dma_start(out=drel_sb[:], in_=drel[:])
            dwts_sb = stpool.tile([P, TC], f32)
            nc.sync.dma_start(out=dwts_sb[:], in_=dwts[:])

            h_sb = hpool.tile([D, rpad], f32)
            sumbuf = tpool.tile([D, NT], f32)
            sqbuf = tpool.tile([D, NT], f32)
            nc.vector.memset(sumbuf[:], 0.0)
            nc.vector.memset(sqbuf[:], 0.0)

            opi = 0
            for rpt in range(repeat_main):
              for t in range(NT):
                agg_ps = apsum.tile([D, P], f32, tag="agg", name=f"agg{t}")
                nchunks_t = sum(nidx // P for (_b, nidx, _kb, _po) in ops_by_tile[t])
                done = 0
                for (bk, nidx, kb, po) in ops_by_tile[t]:
                    g = gpool.tile([128, nidx], f32, tag="g", name=f"g{t}_{kb}")
                    bank_lo = bk * BANK
                    bank_hi = min(bank_lo + BANK, N)
                    nc.gpsimd.dma_gather(
                        g[:].rearrange("p (c e) -> p c e", e=D),
                        feat[bank_lo:bank_hi, :],
                        idx_sb[:, po : po + nidx // 16],
                        nidx, nidx, D,
                        single_packet=True, queue_num=opi % NQ,
                    )
                    opi += 1
                    if stages == 1:
                        nc.vector.tensor_add(out=sumbuf[:, t:t+1], in0=sumbuf[:, t:t+1], in1=g[:, 0:1])
                        done += nidx // P
                        continue
                    for j in range(nidx // P):
                        k = kb + j
                        S = spool.tile([P, P], f32, tag="S", name=f"S{k}")
                        nc.vector.tensor_scalar(
                            out=S[:], in0=iota_sb[:],
                            scalar1=drel_sb[:, k : k + 1],
                            scalar2=dwts_sb[:, k : k + 1],
                            op0=mybir.AluOpType.is_equal,
                            op1=mybir.AluOpType.mult,
                        )
                        if stages >= 3:
                            nc.tensor.matmul(
                                out=agg_ps[:],
                                lhsT=g[:, j * D : (j + 1) * D],
                                rhs=S[:],
                                start=(done == 0), stop=(done == nchunks_t - 1),
                            )
                        else:
                            nc.vector.tensor_add(out=sumbuf[:, t:t+1], in0=sumbuf[:, t:t+1], in1=S[:, 0:1])
                        done += 1

                if stages < 4:
                    continue
                aggT = apool.tile([D, P], f32, tag="aggT", name=f"aggT{t}")
                nc.vector.tensor_copy(out=aggT[:], in_=agg_ps[:])
                hT_ps = hpsum.tile([D, P], f32, tag="hT", name=f"hT{t}")
                nc.tensor.matmul(out=hT_ps[:], lhsT=W_sb[:], rhs=aggT[:],
                                 start=True, stop=True)
                hslice = h_sb[:, t * P : (t + 1) * P]
                nc.scalar.activation(
                    out=hslice, in_=hT_ps[:],
                    func=mybir.ActivationFunctionType.Relu,
                    bias=b_sb[:], accum_out=sumbuf[:, t : t + 1],
                )
                sq = qpool.tile([D, P], f32, tag="sq", name=f"sq{t}")
                nc.scalar.activation(
                    out=sq[:], in_=hslice,
                    func=mybir.ActivationFunctionType.Square,
                    accum_out=sqbuf[:, t : t + 1],
                )

            # BN stats: reduce per-tile partial sums, all-reduce across cores
            if stages < 4:
                nc.vector.tensor_copy(out=h_sb[:, 0:NT], in_=sumbuf[:])
            stats = tpool.tile([D, 2], f32)
            nc.vector.reduce_sum(out=stats[:, 0:1], in_=sumbuf[:], axis=mybir.AxisListType.X)
            nc.vector.reduce_sum(out=stats[:, 1:2], in_=sqbuf[:], axis=mybir.AxisListType.X)
            nc.sync.dma_start(out=cc_in[:], in_=stats[:])
            if use_collective:
                nc.gpsimd.collective_compute(
                    "AllReduce", mybir.AluOpType.add,
                    replica_groups=[list(range(NCORES))],
                    ins=[cc_in[:]], outs=[cc_out[:]],
                )
            else:
                nc.sync.dma_start(out=cc_out[:], in_=cc_in[:])
            gstats = tpool.tile([D, 2], f32)
            nc.sync.dma_start(out=gstats[:], in_=cc_out[:])

            inv_n = 1.0 / float(N)
            mean = tpool.tile([D, 1], f32)
            nc.vector.tensor_scalar_mul(out=mean[:], in0=gstats[:, 0:1], scalar1=inv_n)
            ex2 = tpool.tile([D, 1], f32)
            nc.vector.tensor_scalar_mul(out=ex2[:], in0=gstats[:, 1:2], scalar1=inv_n)
            m2 = tpool.tile([D, 1], f32)
            nc.vector.tensor_mul(out=m2[:], in0=mean[:], in1=mean[:])
            vare = tpool.tile([D, 1], f32)
            nc.vector.tensor_sub(out=vare[:], in0=ex2[:], in1=m2[:])
            nc.vector.tensor_scalar_add(out=vare[:], in0=vare[:], scalar1=BN_EPS)
            rvar = tpool.tile([D, 1], f32)
            nc.vector.reciprocal(out=rvar[:], in_=vare[:])
            rstd = tpool.tile([D, 1], f32)
            nc.scalar.activation(out=rstd[:], in_=rvar[:],
                                 func=mybir.ActivationFunctionType.Sqrt)
            Avec = tpool.tile([D, 1], f32)
            nc.vector.tensor_mul(out=Avec[:], in0=g_sb[:], in1=rstd[:])
            mA = tpool.tile([D, 1], f32)
            nc.vector.tensor_mul(out=mA[:], in0=mean[:], in1=Avec[:])
            Bvec = tpool.tile([D, 1], f32)
            nc.vector.tensor_sub(out=Bvec[:], in0=bt_sb[:], in1=mA[:])

            # apply BN + dropout mask, transpose back, write out
            for rpt in range(repeat_main if stages >= 5 else 0):
              for t in range(NT):
                yt = ypool.tile([D, P], f32, tag="yt", name=f"yt{t}")
                nc.vector.tensor_scalar(
                    out=yt[:], in0=h_sb[:, t * P : (t + 1) * P],
                    scalar1=Avec[:], scalar2=Bvec[:],
                    op0=mybir.AluOpType.mult, op1=mybir.AluOpType.add,
                )
                mk = mpool.tile([D, P], f32, tag="mk", name=f"mk{t}")
                nc.sync.dma_start(out=mk[:], in_=maskT[:, t * P : (t + 1) * P])
                nc.vector.tensor_mul(out=yt[:], in0=yt[:], in1=mk[:])
                tp = tpsum.tile([P, D], f32, tag="tp", name=f"tp{t}")
                nc.tensor.transpose(out=tp[:], in_=yt[:], identity=identity[:])
                yo = ypool.tile([P, D], f32, tag="yo", name=f"yo{t}")
                nc.scalar.copy(out=yo[:], in_=tp[:])
                nc.sync.dma_start(out=yout[t * P : (t + 1) * P, :], in_=yo[:])

    nc.compile()
    return nc


def kernel(feat, src, dst, W, b, gamma, beta):
    feat = np.ascontiguousarray(np.asarray(feat, np.float32))
    src = np.asarray(src).astype(np.int64)
    dst = np.asarray(dst).astype(np.int64)
    N, D = feat.shape
    assert D == 128 and N % NCORES == 0

    meta, idx_all, rel_all, wts_all = _host_prep(feat, src, dst)
    rpc, NT = meta["rpc"], meta["NT"]
    rpad = NT * P

    # dropout mask (deterministic, matches reference)
    import jax
    with jax.default_device(jax.devices("cpu")[0]):
        keep = jax.random.bernoulli(jax.random.key(42), 0.5, (N, D))
        maskv = (np.asarray(keep).astype(np.float32)) * 2.0

    nc = _build_program(meta)

    in_maps = []
    for c in range(NCORES):
        mrows = np.zeros((rpad, D), np.float32)
        mrows[:rpc] = maskv[c * rpc : (c + 1) * rpc]
        in_maps.append({
            "feat": feat,
            "idxs": np.ascontiguousarray(idx_all[c]),
            "drel": np.ascontiguousarray(rel_all[c]),
            "dwts": np.ascontiguousarray(wts_all[c]),
            "maskT": np.ascontiguousarray(mrows.T),
            "W": np.ascontiguousarray(np.asarray(W, np.float32)),
            "b": np.ascontiguousarray(np.asarray(b, np.float32)),
            "gamma": np.ascontiguousarray(np.asarray(gamma, np.float32)),
            "beta": np.ascontiguousarray(np.asarray(beta, np.float32)),
        })

    res = run_bass_kernel_spmd(nc, in_maps, list(range(NCORES)))
    out = np.concatenate([res.results[c]["y"][:rpc] for c in range(NCORES)], axis=0)
    return out
